# revision 32
# baseline (speedup 1.0000x reference)
"""Additive attention (nn_AdditiveAttn) Trainium2 Bass kernel.

Math (reference):
    x = concat([enc, broadcast(cur)], -1)            # (T, B, 2H)
    hid = tanh(x @ W1.T + b1)                        # (T, B, H)
    scores = hid @ W2.T + b2                         # (T, B, 1)
    attn = softmax(scores[..., 0].T, axis=-1)        # (B, T)
    cntxt = einsum('bt,tbh->bh', attn, enc)          # (B, H)
    returns (cntxt, attn)

Kernel strategy: data-parallel over batch B=32 across 8 cores (4 batches
per core). The concat is split: enc @ W1[:, :H].T is the big per-timestep
matmul; cur @ W1[:, H:].T + b1 is a per-batch bias computed once. b2
cancels in the softmax and is dropped. All matmul operands are rounded to
fp32r (fp32 with 11 explicit mantissa bits) which runs the PE at full
bf16 speed; rounding is done host-side.
"""
import sys

sys.path.insert(0, '/opt/trn_rl_repo')

import numpy as np

import concourse.tile as tile
import concourse.mybir as mybir
from concourse import bacc
from concourse.bass_utils import run_bass_kernel_spmd

T = 2048
B = 32
H = 1024
N_CORES = 8
BL = B // N_CORES        # batches per core
KC = H // 128            # 8 contraction chunks
MC = H // 128            # 8 output-row chunks
TB = 2                   # t-blocks per batch
TBS = T // TB            # 1024 columns per t-block
F32 = mybir.dt.float32
F32R = mybir.dt.float32r


def to_fp32r(x: np.ndarray) -> np.ndarray:
    """Round fp32 to fp32r (E8M11: low 12 mantissa bits zeroed, RNE).
    Bit-exact match to the PE's fp32r input rounding (verified on HW)."""
    u = np.ascontiguousarray(x, dtype=np.float32).view(np.uint32).astype(np.uint64)
    lsb = (u >> 12) & 1
    u = u + 0x7FF + lsb
    return np.ascontiguousarray((u & 0xFFFF_F000).astype(np.uint32)).view(np.float32)


def build_kernel():
    nc = bacc.Bacc(trn_type="TRN2", target_bir_lowering=False, debug=False)

    encT = nc.dram_tensor("encT", [BL, H, T], F32R, kind="ExternalInput")
    w1eT = nc.dram_tensor("w1eT", [H, H], F32R, kind="ExternalInput")
    w1cT = nc.dram_tensor("w1cT", [H, H], F32R, kind="ExternalInput")
    curT = nc.dram_tensor("curT", [H, BL], F32R, kind="ExternalInput")
    b1r = nc.dram_tensor("b1r", [1, H], F32R, kind="ExternalInput")
    w2T = nc.dram_tensor("w2T", [H, 1], F32R, kind="ExternalInput")
    ones = nc.dram_tensor("ones", [1, BL], F32R, kind="ExternalInput")
    ctx_out = nc.dram_tensor("ctx_out", [BL, H], F32, kind="ExternalOutput")
    attn_out = nc.dram_tensor("attn_out", [BL, T], F32, kind="ExternalOutput")

    with tile.TileContext(nc) as tc:
        with (
            tc.tile_pool(name="consts", bufs=1) as consts,
            tc.tile_pool(name="encp", bufs=2) as encp,
            tc.tile_pool(name="hidp", bufs=4) as hidp,
            tc.tile_pool(name="smallp", bufs=1) as smallp,
            tc.tile_pool(name="bcp", bufs=2) as bcp,
            tc.tile_pool(name="scrp", bufs=1) as scrp,
            tc.tile_pool(name="ctxp", bufs=2) as ctxp,
            tc.tile_pool(name="mmps", bufs=4, space="PSUM") as mmps,
            tc.tile_pool(name="scps", bufs=2, space="PSUM") as scps,
        ):
            # ---- load weights/constants + batch-0 enc, ordered by
            # first-need time (the DMA engines are bandwidth-serial, so
            # issue order ~= arrival order) ----
            w1e_sb = consts.tile([128, KC, H], F32R)
            w1eT_r = w1eT.rearrange("(c p) n -> p c n", p=128)
            # w1cT shares the second enc slot (released before batch 1's DMA)
            w1c_sb = encp.tile([128, KC, H], F32R, tag="enc")
            w1cT_r = w1cT.rearrange("(c p) n -> p c n", p=128)
            enc_b0 = encp.tile([128, KC, T], F32R, tag="enc")
            encT_b0 = encT[0].rearrange("(c p) t -> p c t", p=128)

            def dma_w1c(mlo, mhi):
                nc.sync.dma_start(
                    w1c_sb[:, :, mlo * 128:mhi * 128],
                    w1cT_r[:, :, mlo * 128:mhi * 128])

            def dma_w1e(mlo, mhi):
                nc.sync.dma_start(
                    w1e_sb[:, :, mlo * 128:mhi * 128],
                    w1eT_r[:, :, mlo * 128:mhi * 128])

            def dma_enc0(lo, hi):
                for kc in range(KC):
                    nc.sync.dma_start(
                        enc_b0[:, kc, lo:hi], encT_b0[:, kc, lo:hi])

            # tiny tensors first (all needed by the earliest PE work)
            # cur gates the very first bias matmul: tiny, goes first on the
            # fast HWDGE queue; the other small tensors ride SWDGE so they
            # don't serialize ahead of the big weight/enc transfers
            cur_sb = consts.tile([128, KC, BL], F32R)
            nc.sync.dma_start(
                cur_sb[:], curT.rearrange("(c p) b -> p c b", p=128))
            b1_full = smallp.tile([1, H * 2], F32R, tag="scores")
            b1_sb = b1_full[:, :H]
            nc.gpsimd.dma_start(b1_sb[:], b1r[:])
            w2_sb = consts.tile([128, MC, 1], F32R)
            nc.gpsimd.dma_start(
                w2_sb[:], w2T.rearrange("(c p) o -> p c o", p=128))
            ones_sb = consts.tile([1, BL], F32R)
            nc.gpsimd.dma_start(ones_sb[:], ones[:])
            def dma_enc0_k(lo, hi, ks):
                for kc in ks:
                    nc.sync.dma_start(
                        enc_b0[:, kc, lo:hi], encT_b0[:, kc, lo:hi])

            dma_w1c(0, 1)
            dma_w1e(0, 1)
            dma_enc0_k(0, 512, range(0, 4))
            dma_w1c(1, 2)
            dma_w1e(1, 2)
            dma_enc0_k(0, 512, range(4, 8))
            for _m in range(2, MC):
                dma_w1c(_m, _m + 1)
                dma_w1e(_m, _m + 1)
            dma_enc0(512, 1024)
            dma_enc0(1024, 1536)
            dma_enc0(1536, 2048)

            # ---- bias[b, h_out] = cur @ W1cur.T + b1, transposed layout ----
            # biasT_sb[p, m, b] = bias for h_out = m*128 + p
            biasT_sb = consts.tile([128, MC, BL], F32)
            for m in range(MC):
                bps = mmps.tile([128, BL], F32, tag="mm")
                for kc in range(KC):
                    nc.tensor.matmul(
                        bps[:], w1c_sb[:, kc, m * 128:(m + 1) * 128],
                        cur_sb[:, kc, :], start=(kc == 0), stop=False)
                nc.tensor.matmul(
                    bps[:], b1_sb[0:1, m * 128:(m + 1) * 128], ones_sb[0:1, :],
                    start=False, stop=True)
                nc.vector.tensor_copy(biasT_sb[:, m, :], bps[:])

            # ---- main loop over local batches ----
            for b in range(BL):
                if b == 0:
                    enc_b = enc_b0
                else:
                    enc_b = encp.tile([128, KC, T], F32R, tag="enc")
                    encT_b = encT[b].rearrange("(c p) t -> p c t", p=128)
                    for tb in range(TB):
                        for kc in range(KC):
                            sl = slice(tb * TBS, (tb + 1) * TBS)
                            nc.sync.dma_start(
                                enc_b[:, kc, sl], encT_b[:, kc, sl])

                scores_b = smallp.tile([1, T], F32, tag="scores")
                sump = smallp.tile([1, 4], F32, tag="sump")
                ctxparts = ctxp.tile([128, KC, 4], F32, tag="ctxparts")
                # staircase: batch 0 runs 512-wide blocks so PE work
                # unlocks in step with the streaming enc/w1e DMAs; the last
                # batch ramps out with small blocks to shorten the exposed
                # softmax+context tail. Matmul N stays >=256 for full fp32r
                # PE rate.
                if b == 0:
                    blocks = [(o, 512) for o in range(0, T, 512)]
                elif b < BL - 1:
                    blocks = [(0, TBS), (TBS, TBS)]
                else:
                    blocks = [(0, TBS), (TBS, 512), (1536, 256), (1792, 256)]
                piece = 0
                for t0, tsz in blocks:
                    sl = slice(t0, t0 + tsz)
                    spans = [(o, min(512, tsz - o)) for o in range(0, tsz, 512)]
                    sc_ps = scps.tile([1, TBS], F32, tag="scps")
                    # score matvecs are emitted one m-group late so the PE
                    # never waits on the tanh that produces its hid input —
                    # except in the very last block, where deferral would
                    # lengthen the exposed softmax+context tail
                    defer = not (b == BL - 1 and (t0, tsz) == blocks[-1])
                    pending = []
                    for m in range(MC):
                        hids = []
                        for o, w in spans:
                            ps = mmps.tile([128, 512], F32, tag="mm")
                            for kc in range(KC):
                                nc.tensor.matmul(
                                    ps[:, :w],
                                    w1e_sb[:, kc, m * 128:(m + 1) * 128],
                                    enc_b[:, kc, t0 + o:t0 + o + w],
                                    start=(kc == 0), stop=(kc == KC - 1))
                            hid = hidp.tile([128, 512], F32R, tag="hid")
                            nc.scalar.activation(
                                hid[:, :w], ps[:, :w],
                                mybir.ActivationFunctionType.Tanh,
                                bias=biasT_sb[:, m, b:b + 1])
                            hids.append((o, w, hid))
                        if defer:
                            for pm, po, pw, phid in pending:
                                nc.tensor.matmul(
                                    sc_ps[:, po:po + pw],
                                    w2_sb[:, pm, :],
                                    phid[:, :pw],
                                    start=(pm == 0), stop=False)
                            pending = [(m, o, w, hid) for (o, w, hid) in hids]
                        else:
                            for o, w, hid in hids:
                                nc.tensor.matmul(
                                    sc_ps[:, o:o + w],
                                    w2_sb[:, m, :],
                                    hid[:, :w],
                                    start=(m == 0), stop=(m == MC - 1))
                    for pm, po, pw, phid in pending:
                        nc.tensor.matmul(
                            sc_ps[:, po:po + pw],
                            w2_sb[:, pm, :],
                            phid[:, :pw],
                            start=(pm == 0), stop=(pm == MC - 1))
                    # online (unnormalized) softmax + context: exp reads
                    # the scores straight out of PSUM (no separate copy;
                    # raw scores are never needed). Scores bounded ~±33 so
                    # exp without max subtraction is safe; b2 is a constant
                    # shift and cancels in the softmax.
                    # (tensor_tensor_reduce crashes TRN2 HW, so mul+reduce.)
                    for o2 in range(0, tsz, TBS):
                        w2sz = min(TBS, tsz - o2)
                        psl = slice(t0 + o2, t0 + o2 + w2sz)
                        nc.scalar.activation(
                            scores_b[:, psl], sc_ps[:, o2:o2 + w2sz],
                            mybir.ActivationFunctionType.Exp,
                            accum_out=sump[:, piece:piece + 1])
                        attn_bc = bcp.tile([128, TBS], F32, tag="bc")
                        nc.gpsimd.partition_broadcast(
                            attn_bc[:, :w2sz], scores_b[:, psl])
                        for kc in range(KC):
                            # last batch: offload some muls to the idle
                            # GPSIMD (own scratch tag; ~2x slower but fully
                            # parallel) to shorten the exposed tail
                            if b == BL - 1 and kc >= 5:
                                scr = scrp.tile([128, TBS], F32, tag="scrg")
                                nc.gpsimd.tensor_mul(
                                    scr[:, :w2sz],
                                    enc_b[:, kc, psl].bitcast(F32),
                                    attn_bc[:, :w2sz])
                            else:
                                scr = scrp.tile([128, TBS], F32, tag="scr")
                                nc.vector.tensor_mul(
                                    scr[:, :w2sz],
                                    enc_b[:, kc, psl].bitcast(F32),
                                    attn_bc[:, :w2sz])
                            nc.vector.tensor_reduce(
                                ctxparts[:, kc, piece:piece + 1],
                                scr[:, :w2sz],
                                axis=mybir.AxisListType.X,
                                op=mybir.AluOpType.add)
                        piece += 1

                # finalize: normalize attn and context by 1/sum(exp)
                nblk = piece
                sumexp = smallp.tile([1, 1], F32, tag="sumexp")
                nc.vector.tensor_reduce(
                    sumexp[:], sump[:, :nblk], axis=mybir.AxisListType.X,
                    op=mybir.AluOpType.add)
                recip = smallp.tile([1, 1], F32, tag="recip")
                nc.vector.reciprocal(recip[:], sumexp[:])
                for hh in range(TB):
                    hsl = slice(hh * TBS, (hh + 1) * TBS)
                    nc.scalar.mul(
                        scores_b[:, hsl], scores_b[:, hsl], mul=recip[:])
                    nc.sync.dma_start(
                        attn_out[b:b + 1, hsl], scores_b[:, hsl])
                recip_bc = ctxp.tile([128, 1], F32, tag="recipbc")
                nc.gpsimd.partition_broadcast(recip_bc[:], recip[:])
                ctxs = ctxp.tile([128, KC], F32, tag="ctxs")
                nc.vector.tensor_reduce(
                    ctxs[:], ctxparts[:, :, :nblk], axis=mybir.AxisListType.X,
                    op=mybir.AluOpType.add)
                ctx = ctxp.tile([128, KC], F32, tag="ctx")
                nc.vector.tensor_scalar_mul(
                    ctx[:], in0=ctxs[:], scalar1=recip_bc[:])
                nc.sync.dma_start(
                    ctx_out[b].rearrange("(c p) -> p c", p=128), ctx[:])

    nc.compile()
    return nc


_NC_CACHE = None


def _get_nc():
    global _NC_CACHE
    if _NC_CACHE is None:
        _NC_CACHE = build_kernel()
    return _NC_CACHE


def kernel(**inputs) -> tuple[np.ndarray, np.ndarray]:
    enc = np.asarray(inputs["encoder_output"], dtype=np.float32)  # (T, B, H)
    cur = np.asarray(inputs["cur_hidden"], dtype=np.float32)      # (B, H)
    W1 = np.asarray(inputs["W1"], dtype=np.float32)               # (H, 2H)
    b1 = np.asarray(inputs["b1"], dtype=np.float32)               # (H,)
    W2 = np.asarray(inputs["W2"], dtype=np.float32)               # (1, H)

    nc = _get_nc()

    # Shared (replicated) weight tensors
    w1eT = to_fp32r(W1[:, :H].T)                 # (H_in, H_out)
    w1cT = to_fp32r(W1[:, H:].T)                 # (H_in, H_out)
    b1r = to_fp32r(b1.reshape(1, H))
    w2T = to_fp32r(W2.reshape(1, H).T)           # (H, 1)
    ones = np.ones((1, BL), dtype=np.float32)    # exact in fp32r

    # (T, B, H) -> (B, H, T), then per-core batch shards
    encT_full = to_fp32r(np.ascontiguousarray(enc.transpose(1, 2, 0)))
    curT_full = to_fp32r(cur.T)                  # (H, B)

    in_maps = []
    for c in range(N_CORES):
        bs = slice(c * BL, (c + 1) * BL)
        in_maps.append({
            "encT": np.ascontiguousarray(encT_full[bs]),
            "w1eT": w1eT,
            "w1cT": w1cT,
            "curT": np.ascontiguousarray(curT_full[:, bs]),
            "b1r": b1r,
            "w2T": w2T,
            "ones": ones,
        })

    res = run_bass_kernel_spmd(nc, in_maps, core_ids=list(range(N_CORES)))
    cntxt = np.concatenate([r["ctx_out"] for r in res.results], axis=0)
    attn = np.concatenate([r["attn_out"] for r in res.results], axis=0)
    return cntxt, attn


# revision 33
# speedup vs baseline: 1.0191x; 1.0191x over previous
"""Additive attention (nn_AdditiveAttn) Trainium2 Bass kernel.

Math (reference):
    x = concat([enc, broadcast(cur)], -1)            # (T, B, 2H)
    hid = tanh(x @ W1.T + b1)                        # (T, B, H)
    scores = hid @ W2.T + b2                         # (T, B, 1)
    attn = softmax(scores[..., 0].T, axis=-1)        # (B, T)
    cntxt = einsum('bt,tbh->bh', attn, enc)          # (B, H)
    returns (cntxt, attn)

Kernel strategy: data-parallel over batch B=32 across 8 cores (4 batches
per core). The concat is split: enc @ W1[:, :H].T is the big per-timestep
matmul; cur @ W1[:, H:].T + b1 is a per-batch bias computed once. b2
cancels in the softmax and is dropped. All matmul operands are rounded to
fp32r (fp32 with 11 explicit mantissa bits) which runs the PE at full
bf16 speed; rounding is done host-side.
"""
import sys

sys.path.insert(0, '/opt/trn_rl_repo')

import numpy as np

import concourse.tile as tile
import concourse.mybir as mybir
from concourse import bacc
from concourse.bass_utils import run_bass_kernel_spmd

T = 2048
B = 32
H = 1024
N_CORES = 8
BL = B // N_CORES        # batches per core
KC = H // 128            # 8 contraction chunks
MC = H // 128            # 8 output-row chunks
TB = 2                   # t-blocks per batch
TBS = T // TB            # 1024 columns per t-block
F32 = mybir.dt.float32
F32R = mybir.dt.float32r


def to_fp32r(x: np.ndarray) -> np.ndarray:
    """Round fp32 to fp32r (E8M11: low 12 mantissa bits zeroed, RNE).
    Bit-exact match to the PE's fp32r input rounding (verified on HW)."""
    u = np.ascontiguousarray(x, dtype=np.float32).view(np.uint32).astype(np.uint64)
    lsb = (u >> 12) & 1
    u = u + 0x7FF + lsb
    return np.ascontiguousarray((u & 0xFFFF_F000).astype(np.uint32)).view(np.float32)


def build_kernel():
    nc = bacc.Bacc(trn_type="TRN2", target_bir_lowering=False, debug=False)

    encT = nc.dram_tensor("encT", [BL, H, T], F32R, kind="ExternalInput")
    w1eT = nc.dram_tensor("w1eT", [H, H], F32R, kind="ExternalInput")
    w1cT = nc.dram_tensor("w1cT", [H, H], F32R, kind="ExternalInput")
    curT = nc.dram_tensor("curT", [H, BL], F32R, kind="ExternalInput")
    b1r = nc.dram_tensor("b1r", [1, H], F32R, kind="ExternalInput")
    w2T = nc.dram_tensor("w2T", [H, 1], F32R, kind="ExternalInput")
    ones = nc.dram_tensor("ones", [1, BL], F32R, kind="ExternalInput")
    ctx_out = nc.dram_tensor("ctx_out", [BL, H], F32, kind="ExternalOutput")
    attn_out = nc.dram_tensor("attn_out", [BL, T], F32, kind="ExternalOutput")

    with tile.TileContext(nc) as tc:
        with (
            tc.tile_pool(name="consts", bufs=1) as consts,
            tc.tile_pool(name="encp", bufs=2) as encp,
            tc.tile_pool(name="hidp", bufs=4) as hidp,
            tc.tile_pool(name="smallp", bufs=1) as smallp,
            tc.tile_pool(name="bcp", bufs=2) as bcp,
            tc.tile_pool(name="scrp", bufs=1) as scrp,
            tc.tile_pool(name="ctxp", bufs=2) as ctxp,
            tc.tile_pool(name="mmps", bufs=4, space="PSUM") as mmps,
            tc.tile_pool(name="scps", bufs=2, space="PSUM") as scps,
        ):
            # ---- load weights/constants + batch-0 enc, ordered by
            # first-need time (the DMA engines are bandwidth-serial, so
            # issue order ~= arrival order) ----
            w1e_sb = consts.tile([128, KC, H], F32R)
            w1eT_r = w1eT.rearrange("(c p) n -> p c n", p=128)
            # w1cT shares the second enc slot (released before batch 1's DMA)
            w1c_sb = encp.tile([128, KC, H], F32R, tag="enc")
            w1cT_r = w1cT.rearrange("(c p) n -> p c n", p=128)
            enc_b0 = encp.tile([128, KC, T], F32R, tag="enc")
            encT_b0 = encT[0].rearrange("(c p) t -> p c t", p=128)

            def dma_w1c(mlo, mhi):
                nc.sync.dma_start(
                    w1c_sb[:, :, mlo * 128:mhi * 128],
                    w1cT_r[:, :, mlo * 128:mhi * 128])

            def dma_w1e(mlo, mhi):
                nc.sync.dma_start(
                    w1e_sb[:, :, mlo * 128:mhi * 128],
                    w1eT_r[:, :, mlo * 128:mhi * 128])

            def dma_enc0(lo, hi):
                for kc in range(KC):
                    nc.sync.dma_start(
                        enc_b0[:, kc, lo:hi], encT_b0[:, kc, lo:hi])

            # tiny tensors first (all needed by the earliest PE work)
            # cur gates the very first bias matmul: tiny, goes first on the
            # fast HWDGE queue; the other small tensors ride SWDGE so they
            # don't serialize ahead of the big weight/enc transfers
            cur_sb = consts.tile([128, KC, BL], F32R)
            nc.sync.dma_start(
                cur_sb[:], curT.rearrange("(c p) b -> p c b", p=128))
            b1_full = smallp.tile([1, H * 2], F32R, tag="scores")
            b1_sb = b1_full[:, :H]
            nc.gpsimd.dma_start(b1_sb[:], b1r[:])
            w2_sb = consts.tile([128, MC, 1], F32R)
            nc.gpsimd.dma_start(
                w2_sb[:], w2T.rearrange("(c p) o -> p c o", p=128))
            ones_sb = consts.tile([1, BL], F32R)
            nc.gpsimd.dma_start(ones_sb[:], ones[:])
            def dma_enc0_k(lo, hi, ks):
                for kc in ks:
                    nc.sync.dma_start(
                        enc_b0[:, kc, lo:hi], encT_b0[:, kc, lo:hi])

            dma_w1c(0, 1)
            dma_w1e(0, 1)
            dma_enc0_k(0, 512, range(0, 4))
            dma_w1c(1, 2)
            dma_w1e(1, 2)
            dma_enc0_k(0, 512, range(4, 8))
            for _m in range(2, MC):
                dma_w1c(_m, _m + 1)
                dma_w1e(_m, _m + 1)
            dma_enc0(512, 1024)
            dma_enc0(1024, 1536)
            dma_enc0(1536, 2048)

            # ---- bias[b, h_out] = cur @ W1cur.T + b1, transposed layout ----
            # biasT_sb[p, m, b] = bias for h_out = m*128 + p
            biasT_sb = consts.tile([128, MC, BL], F32)
            for m in range(MC):
                bps = mmps.tile([128, BL], F32, tag="mm")
                for kc in range(KC):
                    nc.tensor.matmul(
                        bps[:], w1c_sb[:, kc, m * 128:(m + 1) * 128],
                        cur_sb[:, kc, :], start=(kc == 0), stop=False)
                nc.tensor.matmul(
                    bps[:], b1_sb[0:1, m * 128:(m + 1) * 128], ones_sb[0:1, :],
                    start=False, stop=True)
                nc.vector.tensor_copy(biasT_sb[:, m, :], bps[:])

            # ---- main loop over local batches ----
            for b in range(BL):
                if b == 0:
                    enc_b = enc_b0
                else:
                    enc_b = encp.tile([128, KC, T], F32R, tag="enc")
                    encT_b = encT[b].rearrange("(c p) t -> p c t", p=128)
                    for tb in range(TB):
                        for kc in range(KC):
                            sl = slice(tb * TBS, (tb + 1) * TBS)
                            nc.sync.dma_start(
                                enc_b[:, kc, sl], encT_b[:, kc, sl])

                scores_b = smallp.tile([1, T], F32, tag="scores")
                sump = smallp.tile([1, 4], F32, tag="sump")
                ctxparts = ctxp.tile([128, KC, 4], F32, tag="ctxparts")
                # staircase: batch 0 runs 512-wide blocks so PE work
                # unlocks in step with the streaming enc/w1e DMAs; the last
                # batch ramps out with small blocks to shorten the exposed
                # softmax+context tail. Matmul N stays >=256 for full fp32r
                # PE rate.
                if b == 0:
                    blocks = [(o, 512) for o in range(0, T, 512)]
                elif b < BL - 1:
                    blocks = [(0, TBS), (TBS, TBS)]
                else:
                    blocks = [(0, TBS), (TBS, 512), (1536, 256), (1792, 256)]
                piece = 0
                for t0, tsz in blocks:
                    sl = slice(t0, t0 + tsz)
                    spans = [(o, min(512, tsz - o)) for o in range(0, tsz, 512)]
                    sc_ps = scps.tile([1, TBS], F32, tag="scps")
                    # score matvecs are emitted one m-group late so the PE
                    # never waits on the tanh that produces its hid input —
                    # except in the very last block, where deferral would
                    # lengthen the exposed softmax+context tail
                    defer = not (b == BL - 1 and (t0, tsz) == blocks[-1])
                    pending = []
                    for m in range(MC):
                        hids = []
                        for o, w in spans:
                            ps = mmps.tile([128, 512], F32, tag="mm")
                            for kc in range(KC):
                                nc.tensor.matmul(
                                    ps[:, :w],
                                    w1e_sb[:, kc, m * 128:(m + 1) * 128],
                                    enc_b[:, kc, t0 + o:t0 + o + w],
                                    start=(kc == 0), stop=(kc == KC - 1))
                            hid = hidp.tile([128, 512], F32R, tag="hid")
                            nc.scalar.activation(
                                hid[:, :w], ps[:, :w],
                                mybir.ActivationFunctionType.Tanh,
                                bias=biasT_sb[:, m, b:b + 1])
                            hids.append((o, w, hid))
                        if defer:
                            for pm, po, pw, phid in pending:
                                nc.tensor.matmul(
                                    sc_ps[:, po:po + pw],
                                    w2_sb[:, pm, :],
                                    phid[:, :pw],
                                    start=(pm == 0), stop=False)
                            pending = [(m, o, w, hid) for (o, w, hid) in hids]
                        else:
                            for o, w, hid in hids:
                                nc.tensor.matmul(
                                    sc_ps[:, o:o + w],
                                    w2_sb[:, m, :],
                                    hid[:, :w],
                                    start=(m == 0), stop=(m == MC - 1))
                    for pm, po, pw, phid in pending:
                        nc.tensor.matmul(
                            sc_ps[:, po:po + pw],
                            w2_sb[:, pm, :],
                            phid[:, :pw],
                            start=(pm == 0), stop=(pm == MC - 1))
                    # online (unnormalized) softmax + context: exp reads
                    # the scores straight out of PSUM (no separate copy;
                    # raw scores are never needed). Scores bounded ~±33 so
                    # exp without max subtraction is safe; b2 is a constant
                    # shift and cancels in the softmax.
                    # (tensor_tensor_reduce crashes TRN2 HW, so mul+reduce.)
                    for o2 in range(0, tsz, TBS):
                        w2sz = min(TBS, tsz - o2)
                        psl = slice(t0 + o2, t0 + o2 + w2sz)
                        nc.scalar.activation(
                            scores_b[:, psl], sc_ps[:, o2:o2 + w2sz],
                            mybir.ActivationFunctionType.Exp,
                            accum_out=sump[:, piece:piece + 1])
                        attn_bc = bcp.tile([128, TBS], F32, tag="bc")
                        nc.gpsimd.partition_broadcast(
                            attn_bc[:, :w2sz], scores_b[:, psl])
                        final_piece = (b == BL - 1
                                       and (t0, tsz) == blocks[-1])
                        if b == BL - 1 and w2sz <= 512:
                            # tail batch, small piece: pack each chunk's
                            # product into a disjoint column range of one
                            # scratch tile (no slot serialization), split
                            # muls DVE/GPSIMD, and for the very last piece
                            # let the now-idle ACT take some reductions
                            nr = TBS // w2sz
                            scrd = scrp.tile([128, TBS], F32, tag="scr")
                            scrgd = scrp.tile([128, TBS], F32, tag="scrg")
                            for kc in range(KC):
                                gp = kc >= KC // 2
                                tl = scrgd if gp else scrd
                                o3 = ((kc % (KC // 2)) % nr) * w2sz
                                dst = tl[:, o3:o3 + w2sz]
                                eng = nc.gpsimd if gp else nc.vector
                                eng.tensor_mul(
                                    dst, enc_b[:, kc, psl].bitcast(F32),
                                    attn_bc[:, :w2sz])
                                acc = ctxparts[:, kc, piece:piece + 1]
                                if final_piece and kc >= 5:
                                    nc.scalar.activation(
                                        dst, dst,
                                        mybir.ActivationFunctionType.Identity,
                                        accum_out=acc)
                                else:
                                    nc.vector.tensor_reduce(
                                        acc, dst, axis=mybir.AxisListType.X,
                                        op=mybir.AluOpType.add)
                        else:
                            for kc in range(KC):
                                if b == BL - 1 and kc >= 5:
                                    scr = scrp.tile([128, TBS], F32,
                                                    tag="scrg")
                                    nc.gpsimd.tensor_mul(
                                        scr[:, :w2sz],
                                        enc_b[:, kc, psl].bitcast(F32),
                                        attn_bc[:, :w2sz])
                                else:
                                    scr = scrp.tile([128, TBS], F32,
                                                    tag="scr")
                                    nc.vector.tensor_mul(
                                        scr[:, :w2sz],
                                        enc_b[:, kc, psl].bitcast(F32),
                                        attn_bc[:, :w2sz])
                                nc.vector.tensor_reduce(
                                    ctxparts[:, kc, piece:piece + 1],
                                    scr[:, :w2sz],
                                    axis=mybir.AxisListType.X,
                                    op=mybir.AluOpType.add)
                        piece += 1

                # finalize: normalize attn and context by 1/sum(exp)
                nblk = piece
                sumexp = smallp.tile([1, 1], F32, tag="sumexp")
                nc.vector.tensor_reduce(
                    sumexp[:], sump[:, :nblk], axis=mybir.AxisListType.X,
                    op=mybir.AluOpType.add)
                recip = smallp.tile([1, 1], F32, tag="recip")
                nc.vector.reciprocal(recip[:], sumexp[:])
                for hh in range(TB):
                    hsl = slice(hh * TBS, (hh + 1) * TBS)
                    nc.scalar.mul(
                        scores_b[:, hsl], scores_b[:, hsl], mul=recip[:])
                    nc.sync.dma_start(
                        attn_out[b:b + 1, hsl], scores_b[:, hsl])
                recip_bc = ctxp.tile([128, 1], F32, tag="recipbc")
                nc.gpsimd.partition_broadcast(recip_bc[:], recip[:])
                ctxs = ctxp.tile([128, KC], F32, tag="ctxs")
                nc.vector.tensor_reduce(
                    ctxs[:], ctxparts[:, :, :nblk], axis=mybir.AxisListType.X,
                    op=mybir.AluOpType.add)
                ctx = ctxp.tile([128, KC], F32, tag="ctx")
                nc.vector.tensor_scalar_mul(
                    ctx[:], in0=ctxs[:], scalar1=recip_bc[:])
                nc.sync.dma_start(
                    ctx_out[b].rearrange("(c p) -> p c", p=128), ctx[:])

    nc.compile()
    return nc


_NC_CACHE = None


def _get_nc():
    global _NC_CACHE
    if _NC_CACHE is None:
        _NC_CACHE = build_kernel()
    return _NC_CACHE


def kernel(**inputs) -> tuple[np.ndarray, np.ndarray]:
    enc = np.asarray(inputs["encoder_output"], dtype=np.float32)  # (T, B, H)
    cur = np.asarray(inputs["cur_hidden"], dtype=np.float32)      # (B, H)
    W1 = np.asarray(inputs["W1"], dtype=np.float32)               # (H, 2H)
    b1 = np.asarray(inputs["b1"], dtype=np.float32)               # (H,)
    W2 = np.asarray(inputs["W2"], dtype=np.float32)               # (1, H)

    nc = _get_nc()

    # Shared (replicated) weight tensors
    w1eT = to_fp32r(W1[:, :H].T)                 # (H_in, H_out)
    w1cT = to_fp32r(W1[:, H:].T)                 # (H_in, H_out)
    b1r = to_fp32r(b1.reshape(1, H))
    w2T = to_fp32r(W2.reshape(1, H).T)           # (H, 1)
    ones = np.ones((1, BL), dtype=np.float32)    # exact in fp32r

    # (T, B, H) -> (B, H, T), then per-core batch shards
    encT_full = to_fp32r(np.ascontiguousarray(enc.transpose(1, 2, 0)))
    curT_full = to_fp32r(cur.T)                  # (H, B)

    in_maps = []
    for c in range(N_CORES):
        bs = slice(c * BL, (c + 1) * BL)
        in_maps.append({
            "encT": np.ascontiguousarray(encT_full[bs]),
            "w1eT": w1eT,
            "w1cT": w1cT,
            "curT": np.ascontiguousarray(curT_full[:, bs]),
            "b1r": b1r,
            "w2T": w2T,
            "ones": ones,
        })

    res = run_bass_kernel_spmd(nc, in_maps, core_ids=list(range(N_CORES)))
    cntxt = np.concatenate([r["ctx_out"] for r in res.results], axis=0)
    attn = np.concatenate([r["attn_out"] for r in res.results], axis=0)
    return cntxt, attn


# revision 34
# speedup vs baseline: 1.0630x; 1.0431x over previous
"""Additive attention (nn_AdditiveAttn) Trainium2 Bass kernel.

Math (reference):
    x = concat([enc, broadcast(cur)], -1)            # (T, B, 2H)
    hid = tanh(x @ W1.T + b1)                        # (T, B, H)
    scores = hid @ W2.T + b2                         # (T, B, 1)
    attn = softmax(scores[..., 0].T, axis=-1)        # (B, T)
    cntxt = einsum('bt,tbh->bh', attn, enc)          # (B, H)
    returns (cntxt, attn)

Kernel strategy: data-parallel over batch B=32 across 8 cores (4 batches
per core). The concat is split: enc @ W1[:, :H].T is the big per-timestep
matmul; cur @ W1[:, H:].T + b1 is a per-batch bias computed once. b2
cancels in the softmax and is dropped. All matmul operands are rounded to
fp32r (fp32 with 11 explicit mantissa bits) which runs the PE at full
bf16 speed; rounding is done host-side.
"""
import sys

sys.path.insert(0, '/opt/trn_rl_repo')

import numpy as np

import concourse.tile as tile
import concourse.mybir as mybir
from concourse import bacc
from concourse.bass_utils import run_bass_kernel_spmd

T = 2048
B = 32
H = 1024
N_CORES = 8
BL = B // N_CORES        # batches per core
KC = H // 128            # 8 contraction chunks
MC = H // 128            # 8 output-row chunks
TB = 2                   # t-blocks per batch
TBS = T // TB            # 1024 columns per t-block
F32 = mybir.dt.float32
F32R = mybir.dt.float32r


def to_fp32r(x: np.ndarray) -> np.ndarray:
    """Round fp32 to fp32r (E8M11: low 12 mantissa bits zeroed, RNE).
    Bit-exact match to the PE's fp32r input rounding (verified on HW)."""
    u = np.ascontiguousarray(x, dtype=np.float32).view(np.uint32).astype(np.uint64)
    lsb = (u >> 12) & 1
    u = u + 0x7FF + lsb
    return np.ascontiguousarray((u & 0xFFFF_F000).astype(np.uint32)).view(np.float32)


def build_kernel():
    nc = bacc.Bacc(trn_type="TRN2", target_bir_lowering=False, debug=False)

    encT = nc.dram_tensor("encT", [BL, H, T], F32R, kind="ExternalInput")
    w1eT = nc.dram_tensor("w1eT", [H, H], F32R, kind="ExternalInput")
    w1cT = nc.dram_tensor("w1cT", [H, H], F32R, kind="ExternalInput")
    curT = nc.dram_tensor("curT", [H, BL], F32R, kind="ExternalInput")
    b1r = nc.dram_tensor("b1r", [1, H], F32R, kind="ExternalInput")
    w2T = nc.dram_tensor("w2T", [H, 1], F32R, kind="ExternalInput")
    ones = nc.dram_tensor("ones", [1, BL], F32R, kind="ExternalInput")
    ctx_out = nc.dram_tensor("ctx_out", [BL, H], F32, kind="ExternalOutput")
    attn_out = nc.dram_tensor("attn_out", [BL, T], F32, kind="ExternalOutput")

    with tile.TileContext(nc) as tc:
        with (
            tc.tile_pool(name="consts", bufs=1) as consts,
            tc.tile_pool(name="encp", bufs=2) as encp,
            tc.tile_pool(name="hidp", bufs=4) as hidp,
            tc.tile_pool(name="smallp", bufs=1) as smallp,
            tc.tile_pool(name="bcp", bufs=2) as bcp,
            tc.tile_pool(name="scrp", bufs=1) as scrp,
            tc.tile_pool(name="ctxp", bufs=2) as ctxp,
            tc.tile_pool(name="mmps", bufs=4, space="PSUM") as mmps,
            tc.tile_pool(name="scps", bufs=2, space="PSUM") as scps,
        ):
            # ---- load weights/constants + batch-0 enc, ordered by
            # first-need time (the DMA engines are bandwidth-serial, so
            # issue order ~= arrival order) ----
            w1e_sb = consts.tile([128, KC, H], F32R)
            w1eT_r = w1eT.rearrange("(c p) n -> p c n", p=128)
            # w1cT shares the second enc slot (released before batch 1's DMA)
            w1c_sb = encp.tile([128, KC, H], F32R, tag="enc")
            w1cT_r = w1cT.rearrange("(c p) n -> p c n", p=128)
            enc_b0 = encp.tile([128, KC, T], F32R, tag="enc")
            encT_b0 = encT[0].rearrange("(c p) t -> p c t", p=128)

            def dma_w1c(mlo, mhi):
                nc.sync.dma_start(
                    w1c_sb[:, :, mlo * 128:mhi * 128],
                    w1cT_r[:, :, mlo * 128:mhi * 128])

            def dma_w1e(mlo, mhi):
                nc.sync.dma_start(
                    w1e_sb[:, :, mlo * 128:mhi * 128],
                    w1eT_r[:, :, mlo * 128:mhi * 128])

            def dma_enc0(lo, hi):
                for kc in range(KC):
                    nc.sync.dma_start(
                        enc_b0[:, kc, lo:hi], encT_b0[:, kc, lo:hi])

            # tiny tensors first (all needed by the earliest PE work)
            # cur gates the very first bias matmul: tiny, goes first on the
            # fast HWDGE queue; the other small tensors ride SWDGE so they
            # don't serialize ahead of the big weight/enc transfers
            cur_sb = consts.tile([128, KC, BL], F32R)
            nc.sync.dma_start(
                cur_sb[:], curT.rearrange("(c p) b -> p c b", p=128))
            b1_full = smallp.tile([1, H * 2], F32R, tag="scores")
            b1_sb = b1_full[:, :H]
            nc.gpsimd.dma_start(b1_sb[:], b1r[:])
            w2_sb = consts.tile([128, MC, 1], F32R)
            nc.gpsimd.dma_start(
                w2_sb[:], w2T.rearrange("(c p) o -> p c o", p=128))
            ones_sb = consts.tile([1, BL], F32R)
            nc.gpsimd.dma_start(ones_sb[:], ones[:])
            def dma_enc0_k(lo, hi, ks):
                for kc in ks:
                    nc.sync.dma_start(
                        enc_b0[:, kc, lo:hi], encT_b0[:, kc, lo:hi])

            dma_w1c(0, 1)
            dma_w1e(0, 1)
            dma_enc0_k(0, 512, range(0, 4))
            dma_w1c(1, 2)
            dma_w1e(1, 2)
            dma_enc0_k(0, 512, range(4, 8))
            for _m in range(2, MC):
                dma_w1c(_m, _m + 1)
                dma_w1e(_m, _m + 1)
            dma_enc0(512, 1024)
            dma_enc0(1024, 1536)
            dma_enc0(1536, 2048)

            # ---- bias[b, h_out] = cur @ W1cur.T + b1, transposed layout ----
            # biasT_sb[p, m, b] = bias for h_out = m*128 + p. Bias group m is
            # emitted interleaved into batch 0's first-block m-loop: the PE is
            # in-order, so emitting all bias groups up front would head-of-line
            # block batch 0's matmuls on the last w1c slice's DMA.
            biasT_sb = consts.tile([128, MC, BL], F32)

            def emit_bias_group(m):
                bps = mmps.tile([128, BL], F32, tag="mm")
                for kc in range(KC):
                    nc.tensor.matmul(
                        bps[:], w1c_sb[:, kc, m * 128:(m + 1) * 128],
                        cur_sb[:, kc, :], start=(kc == 0), stop=False)
                nc.tensor.matmul(
                    bps[:], b1_sb[0:1, m * 128:(m + 1) * 128], ones_sb[0:1, :],
                    start=False, stop=True)
                nc.vector.tensor_copy(biasT_sb[:, m, :], bps[:])

            # ---- main loop over local batches ----
            for b in range(BL):
                if b == 0:
                    enc_b = enc_b0
                else:
                    enc_b = encp.tile([128, KC, T], F32R, tag="enc")
                    encT_b = encT[b].rearrange("(c p) t -> p c t", p=128)
                    for tb in range(TB):
                        for kc in range(KC):
                            sl = slice(tb * TBS, (tb + 1) * TBS)
                            nc.sync.dma_start(
                                enc_b[:, kc, sl], encT_b[:, kc, sl])

                scores_b = smallp.tile([1, T], F32, tag="scores")
                sump = smallp.tile([1, 4], F32, tag="sump")
                ctxparts = ctxp.tile([128, KC, 4], F32, tag="ctxparts")
                # staircase: batch 0 runs 512-wide blocks so PE work
                # unlocks in step with the streaming enc/w1e DMAs; the last
                # batch ramps out with small blocks to shorten the exposed
                # softmax+context tail. Matmul N stays >=256 for full fp32r
                # PE rate.
                if b == 0:
                    blocks = [(o, 512) for o in range(0, T, 512)]
                elif b < BL - 1:
                    blocks = [(0, TBS), (TBS, TBS)]
                else:
                    blocks = [(0, TBS), (TBS, 512), (1536, 256), (1792, 256)]
                piece = 0
                for t0, tsz in blocks:
                    sl = slice(t0, t0 + tsz)
                    spans = [(o, min(512, tsz - o)) for o in range(0, tsz, 512)]
                    sc_ps = scps.tile([1, TBS], F32, tag="scps")
                    # score matvecs are emitted one m-group late so the PE
                    # never waits on the tanh that produces its hid input —
                    # except in the very last block, where deferral would
                    # lengthen the exposed softmax+context tail
                    defer = not (b == BL - 1 and (t0, tsz) == blocks[-1])
                    pending = []
                    for m in range(MC):
                        if b == 0 and t0 == 0:
                            emit_bias_group(m)
                        hids = []
                        for o, w in spans:
                            ps = mmps.tile([128, 512], F32, tag="mm")
                            for kc in range(KC):
                                nc.tensor.matmul(
                                    ps[:, :w],
                                    w1e_sb[:, kc, m * 128:(m + 1) * 128],
                                    enc_b[:, kc, t0 + o:t0 + o + w],
                                    start=(kc == 0), stop=(kc == KC - 1))
                            hid = hidp.tile([128, 512], F32R, tag="hid")
                            nc.scalar.activation(
                                hid[:, :w], ps[:, :w],
                                mybir.ActivationFunctionType.Tanh,
                                bias=biasT_sb[:, m, b:b + 1])
                            hids.append((o, w, hid))
                        if defer:
                            for pm, po, pw, phid in pending:
                                nc.tensor.matmul(
                                    sc_ps[:, po:po + pw],
                                    w2_sb[:, pm, :],
                                    phid[:, :pw],
                                    start=(pm == 0), stop=False)
                            pending = [(m, o, w, hid) for (o, w, hid) in hids]
                        else:
                            for o, w, hid in hids:
                                nc.tensor.matmul(
                                    sc_ps[:, o:o + w],
                                    w2_sb[:, m, :],
                                    hid[:, :w],
                                    start=(m == 0), stop=(m == MC - 1))
                    for pm, po, pw, phid in pending:
                        nc.tensor.matmul(
                            sc_ps[:, po:po + pw],
                            w2_sb[:, pm, :],
                            phid[:, :pw],
                            start=(pm == 0), stop=(pm == MC - 1))
                    # online (unnormalized) softmax + context: exp reads
                    # the scores straight out of PSUM (no separate copy;
                    # raw scores are never needed). Scores bounded ~±33 so
                    # exp without max subtraction is safe; b2 is a constant
                    # shift and cancels in the softmax.
                    # (tensor_tensor_reduce crashes TRN2 HW, so mul+reduce.)
                    for o2 in range(0, tsz, TBS):
                        w2sz = min(TBS, tsz - o2)
                        psl = slice(t0 + o2, t0 + o2 + w2sz)
                        nc.scalar.activation(
                            scores_b[:, psl], sc_ps[:, o2:o2 + w2sz],
                            mybir.ActivationFunctionType.Exp,
                            accum_out=sump[:, piece:piece + 1])
                        attn_bc = bcp.tile([128, TBS], F32, tag="bc")
                        nc.gpsimd.partition_broadcast(
                            attn_bc[:, :w2sz], scores_b[:, psl])
                        final_piece = (b == BL - 1
                                       and (t0, tsz) == blocks[-1])
                        if b == BL - 1 and w2sz <= 512:
                            # tail batch, small piece: pack each chunk's
                            # product into a disjoint column range of one
                            # scratch tile (no slot serialization), split
                            # muls DVE/GPSIMD, and for the very last piece
                            # let the now-idle ACT take some reductions
                            nr = TBS // w2sz
                            scrd = scrp.tile([128, TBS], F32, tag="scr")
                            scrgd = scrp.tile([128, TBS], F32, tag="scrg")
                            for kc in range(KC):
                                gp = kc >= KC // 2
                                tl = scrgd if gp else scrd
                                o3 = ((kc % (KC // 2)) % nr) * w2sz
                                dst = tl[:, o3:o3 + w2sz]
                                eng = nc.gpsimd if gp else nc.vector
                                eng.tensor_mul(
                                    dst, enc_b[:, kc, psl].bitcast(F32),
                                    attn_bc[:, :w2sz])
                                acc = ctxparts[:, kc, piece:piece + 1]
                                if final_piece and kc >= 5:
                                    nc.scalar.activation(
                                        dst, dst,
                                        mybir.ActivationFunctionType.Identity,
                                        accum_out=acc)
                                else:
                                    nc.vector.tensor_reduce(
                                        acc, dst, axis=mybir.AxisListType.X,
                                        op=mybir.AluOpType.add)
                        else:
                            for kc in range(KC):
                                if b == BL - 1 and kc >= 5:
                                    scr = scrp.tile([128, TBS], F32,
                                                    tag="scrg")
                                    nc.gpsimd.tensor_mul(
                                        scr[:, :w2sz],
                                        enc_b[:, kc, psl].bitcast(F32),
                                        attn_bc[:, :w2sz])
                                else:
                                    scr = scrp.tile([128, TBS], F32,
                                                    tag="scr")
                                    nc.vector.tensor_mul(
                                        scr[:, :w2sz],
                                        enc_b[:, kc, psl].bitcast(F32),
                                        attn_bc[:, :w2sz])
                                nc.vector.tensor_reduce(
                                    ctxparts[:, kc, piece:piece + 1],
                                    scr[:, :w2sz],
                                    axis=mybir.AxisListType.X,
                                    op=mybir.AluOpType.add)
                        piece += 1

                # finalize: normalize attn and context by 1/sum(exp)
                nblk = piece
                sumexp = smallp.tile([1, 1], F32, tag="sumexp")
                nc.vector.tensor_reduce(
                    sumexp[:], sump[:, :nblk], axis=mybir.AxisListType.X,
                    op=mybir.AluOpType.add)
                recip = smallp.tile([1, 1], F32, tag="recip")
                nc.vector.reciprocal(recip[:], sumexp[:])
                for hh in range(TB):
                    hsl = slice(hh * TBS, (hh + 1) * TBS)
                    nc.scalar.mul(
                        scores_b[:, hsl], scores_b[:, hsl], mul=recip[:])
                    nc.sync.dma_start(
                        attn_out[b:b + 1, hsl], scores_b[:, hsl])
                recip_bc = ctxp.tile([128, 1], F32, tag="recipbc")
                nc.gpsimd.partition_broadcast(recip_bc[:], recip[:])
                ctxs = ctxp.tile([128, KC], F32, tag="ctxs")
                nc.vector.tensor_reduce(
                    ctxs[:], ctxparts[:, :, :nblk], axis=mybir.AxisListType.X,
                    op=mybir.AluOpType.add)
                ctx = ctxp.tile([128, KC], F32, tag="ctx")
                nc.vector.tensor_scalar_mul(
                    ctx[:], in0=ctxs[:], scalar1=recip_bc[:])
                nc.sync.dma_start(
                    ctx_out[b].rearrange("(c p) -> p c", p=128), ctx[:])

    nc.compile()
    return nc


_NC_CACHE = None


def _get_nc():
    global _NC_CACHE
    if _NC_CACHE is None:
        _NC_CACHE = build_kernel()
    return _NC_CACHE


def kernel(**inputs) -> tuple[np.ndarray, np.ndarray]:
    enc = np.asarray(inputs["encoder_output"], dtype=np.float32)  # (T, B, H)
    cur = np.asarray(inputs["cur_hidden"], dtype=np.float32)      # (B, H)
    W1 = np.asarray(inputs["W1"], dtype=np.float32)               # (H, 2H)
    b1 = np.asarray(inputs["b1"], dtype=np.float32)               # (H,)
    W2 = np.asarray(inputs["W2"], dtype=np.float32)               # (1, H)

    nc = _get_nc()

    # Shared (replicated) weight tensors
    w1eT = to_fp32r(W1[:, :H].T)                 # (H_in, H_out)
    w1cT = to_fp32r(W1[:, H:].T)                 # (H_in, H_out)
    b1r = to_fp32r(b1.reshape(1, H))
    w2T = to_fp32r(W2.reshape(1, H).T)           # (H, 1)
    ones = np.ones((1, BL), dtype=np.float32)    # exact in fp32r

    # (T, B, H) -> (B, H, T), then per-core batch shards
    encT_full = to_fp32r(np.ascontiguousarray(enc.transpose(1, 2, 0)))
    curT_full = to_fp32r(cur.T)                  # (H, B)

    in_maps = []
    for c in range(N_CORES):
        bs = slice(c * BL, (c + 1) * BL)
        in_maps.append({
            "encT": np.ascontiguousarray(encT_full[bs]),
            "w1eT": w1eT,
            "w1cT": w1cT,
            "curT": np.ascontiguousarray(curT_full[:, bs]),
            "b1r": b1r,
            "w2T": w2T,
            "ones": ones,
        })

    res = run_bass_kernel_spmd(nc, in_maps, core_ids=list(range(N_CORES)))
    cntxt = np.concatenate([r["ctx_out"] for r in res.results], axis=0)
    attn = np.concatenate([r["attn_out"] for r in res.results], axis=0)
    return cntxt, attn
